# revision 13
# baseline (speedup 1.0000x reference)
"""Cross-channel multi-head attention on 8 Trainium2 NeuronCores.

Sharding: data-parallel over the batch axis. bs2=16 sequences form bs=8
(batch, 2-channel) pairs; each core handles one pair fully locally
(cross-channel attention couples only the two channels of the same batch
element), so no collectives are needed.

Per core (T=2048 tokens = 2 channels x 1024 patches, D=1024, H=8 heads,
dk=128; heads 0..5 attend to the other channel's K/V, heads 6..7 to the
same channel):
  1. Qt = (x @ Wq + bq)^T and Kt likewise, in [D, T] layout (dk on
     partitions) -- exactly what the scores matmul wants as lhsT/rhs.
  2. V = x @ Wv + bv in natural [T, D] layout, stored per head with an
     extra ones column (softmax denominator trick).
  3. Per (head, channel): S^T[m,n] = Kt_h^T-slice x Qt_h-slice;
     P^T = exp(S^T / sqrt(dk)) (no max subtraction: |S|<~6 for this
     distribution); Z_aug = P^T.T @ [V_h | 1] so Z_aug[:,dk] is the
     denominator; Z = Z_aug[:,:dk] * (1/denom); PE-transpose Z into
     Zt [D, T].
  4. out = Zt^T-slices @ Wo + bo in natural [T, D] layout.

All matmuls in bf16 with f32 PSUM accumulation (~5e-3 rel err vs the f32
reference). The host pre-transposes/casts x to bf16 [D, T] per core and
casts the weights to bf16, so the device does no f32 transposes.
"""

import sys

if "/opt/trn_rl_repo" not in sys.path:
    sys.path.insert(0, "/opt/trn_rl_repo")

import numpy as np
import ml_dtypes

import concourse.bass as bass
import concourse.tile as tile
from concourse import mybir
from concourse.bass_utils import run_bass_kernel_spmd
from concourse.masks import make_identity

# Walrus in this container rejects >1 wait condition on TPB_CTRL ops
# (Tile's kernel-tail drain carries one per active proc). Split them.
import os

_here = os.path.dirname(os.path.abspath(__file__))
if _here not in sys.path:
    sys.path.insert(0, _here)
try:
    import bir_legalize
except ImportError:  # graded in a bare dir: fall back to inline copy
    bir_legalize = None

N = 1024  # patches per channel
D = 1024
H = 8
DK = 128
N_CROSS = 6
T = 2 * N  # tokens per core (2 channels of one batch element)
P = 128
KO = D // P  # 8 outer chunks of the 1024-wide dims
TC = T // P  # 16 token chunks
BF = mybir.dt.bfloat16
F32 = mybir.dt.float32
SCALE = 1.0 / float(np.sqrt(DK))

_CACHE = {}


def _legalize_install():
    if bir_legalize is not None:
        bir_legalize.install()
        return
    # Inline fallback (kernel.py must be self-contained when graded).
    import json
    import concourse.bass2jax as bass2jax
    from concourse.bass_utils import compile_bir_kernel as _orig

    if getattr(bass2jax.compile_bir_kernel, "_legalized", False):
        return

    OPCODE_MAX = {}
    SKIP = set()

    def _legalize(bir_json):
        d = json.loads(bir_json)
        changed = False
        for fn in d.get("functions", []):
            for bb in fn.get("blocks") or fn.get("basicblocks") or []:
                out = []
                for inst in bb.get("instructions", []):
                    sync = inst.get("sync_info") or {}
                    waits = sync.get("on_wait") or []
                    cap = OPCODE_MAX.get(inst.get("opcode"), 1)
                    if len(waits) > cap and inst.get("opcode") not in SKIP:
                        extra, keep = waits[:-cap], waits[-cap:]
                        for i, w in enumerate(extra):
                            out.append(
                                {
                                    "debug": inst.get("debug", 0),
                                    "engine": inst["engine"],
                                    "ins": [],
                                    "outs": [],
                                    "is_reset_sema": False,
                                    "name": f"{inst['name']}-sw{i}",
                                    "opcode": "Drain",
                                    "sync_info": {"on_update": [], "on_wait": [w]},
                                }
                            )
                        sync["on_wait"] = keep
                        inst["sync_info"] = sync
                        changed = True
                    out.append(inst)
                bb["instructions"] = out
        return json.dumps(d).encode() if changed else bir_json

    def compile_bir_kernel(bir_json, tmpdir, neff_name="file.neff"):
        return _orig(_legalize(bir_json), tmpdir, neff_name)

    compile_bir_kernel._legalized = True
    bass2jax.compile_bir_kernel = compile_bir_kernel


def _bcast_rows(ap, p):
    """Replicate a 1-D DRAM AP across p partitions (stride-0 partition dim)."""
    return bass.AP(tensor=ap.tensor, offset=ap.offset, ap=[[0, p], *ap.ap])


def _build():
    nc = bass.Bass()

    xt_d = nc.dram_tensor("xt", [D, T], BF, kind="ExternalInput").ap()
    wq_d = nc.dram_tensor("wq", [D, D], BF, kind="ExternalInput").ap()
    wk_d = nc.dram_tensor("wk", [D, D], BF, kind="ExternalInput").ap()
    wv_d = nc.dram_tensor("wv", [D, D], BF, kind="ExternalInput").ap()
    wo_d = nc.dram_tensor("wo", [D, D], BF, kind="ExternalInput").ap()
    bq_d = nc.dram_tensor("bq", [D], F32, kind="ExternalInput").ap()
    bk_d = nc.dram_tensor("bk", [D], F32, kind="ExternalInput").ap()
    bv_d = nc.dram_tensor("bv", [D], F32, kind="ExternalInput").ap()
    bo_d = nc.dram_tensor("bo", [D], F32, kind="ExternalInput").ap()
    out_d = nc.dram_tensor("out", [T, D], F32, kind="ExternalOutput").ap()

    with tile.TileContext(nc) as tc:
        with (
            tc.tile_pool(name="consts", bufs=1) as consts,
            tc.tile_pool(name="big", bufs=1) as big,
        ):
            ident = consts.tile([P, P], BF)
            make_identity(nc, ident)
            bq_p = consts.tile([P, KO], F32)
            nc.sync.dma_start(bq_p[:], bq_d.rearrange("(o p) -> p o", p=P))
            bk_p = consts.tile([P, KO], F32)
            nc.sync.dma_start(bk_p[:], bk_d.rearrange("(o p) -> p o", p=P))
            bv_r = consts.tile([P, D], F32)
            nc.sync.dma_start(bv_r[:], _bcast_rows(bv_d, P))
            bo_r = consts.tile([P, D], F32)
            nc.sync.dma_start(bo_r[:], _bcast_rows(bo_d, P))

            Vg = big.tile([P, TC, H, DK + 1], BF)  # natural V + ones col
            nc.vector.memset(Vg[:, :, :, DK : DK + 1], 1.0)
            Zt = big.tile([P, KO, T], BF)  # attention out, [dout, T]

            # ---- phases A (V proj) + B (QK proj fused with attention) ----
            with (
                tc.tile_pool(name="xt_w", bufs=1) as xt_w,
                tc.tile_pool(name="qk", bufs=2) as qk,
                tc.tile_pool(name="pt_pool", bufs=2) as pt_pool,
                tc.tile_pool(name="att_sm", bufs=4) as att_sm,
                tc.tile_pool(name="ps1", bufs=2, space="PSUM") as ps1,
                tc.tile_pool(name="ps_s", bufs=2, space="PSUM") as ps_s,
                tc.tile_pool(name="ps_z", bufs=2, space="PSUM") as ps_z,
                tc.tile_pool(name="ps_zt", bufs=2, space="PSUM") as ps_zt,
            ):
                # Warm the PE HAM clock gate with throwaway accumulating
                # matmul groups (dense, no psum rotation stalls) so the
                # first real matmuls run at 2.4 GHz instead of 1.2 GHz.
                warm_in = att_sm.tile([P, P], BF, tag="warm")
                nc.vector.memset(warm_in[:], 0.0)
                warm_rhs = att_sm.tile([P, 512], BF, tag="warm_rhs")
                nc.vector.memset(warm_rhs[:], 0.0)
                for g in range(2):
                    wps = ps_s.tile([P, 512], F32, tag="s")
                    for k in range(12):
                        nc.tensor.matmul(
                            wps[:],
                            warm_in[:],
                            warm_rhs[:],
                            start=(k == 0),
                            stop=(k == 11),
                        )

                # DMA: V-projection inputs (Xt, Wv) issue on sync, Wq/Wk on
                # gpsimd in parallel (descriptor issue is the startup
                # bottleneck). First k-chunks split fine across queues so the
                # first accumulation group can start ASAP.
                Xt = xt_w.tile([P, KO, T], BF)
                Wq = xt_w.tile([P, KO, D], BF)
                Wk = xt_w.tile([P, KO, D], BF)
                Wv = xt_w.tile([P, KO, D], BF)
                xt_r = xt_d.rearrange("(o p) t -> p o t", p=P)
                wq_r = wq_d.rearrange("(o p) f -> p o f", p=P)
                wk_r = wk_d.rearrange("(o p) f -> p o f", p=P)
                wv_r = wv_d.rearrange("(o p) f -> p o f", p=P)
                for q in range(4):
                    nc.sync.dma_start(
                        Xt[:, 0, q * 512 : (q + 1) * 512],
                        xt_r[:, 0, q * 512 : (q + 1) * 512],
                    )
                nc.sync.dma_start(Wv[:, 0, :512], wv_r[:, 0, :512])
                nc.sync.dma_start(Wv[:, 0, 512:], wv_r[:, 0, 512:])
                for o in range(1, KO):
                    nc.sync.dma_start(Xt[:, o, :], xt_r[:, o, :])
                    nc.sync.dma_start(Wv[:, o, :], wv_r[:, o, :])
                for o in range(KO):
                    nc.gpsimd.dma_start(Wq[:, o, :], wq_r[:, o, :])
                    nc.gpsimd.dma_start(Wk[:, o, :], wk_r[:, o, :])

                # phase A: V natural = Xt-chunk.T @ Wv
                for tci in range(TC):
                    for dh in range(2):
                        ps = ps1.tile([P, 512], F32, tag="ps1")
                        for k in range(KO):
                            nc.tensor.matmul(
                                ps[:],
                                Xt[:, k, tci * P : (tci + 1) * P],
                                Wv[:, k, dh * 512 : (dh + 1) * 512],
                                start=(k == 0),
                                stop=(k == KO - 1),
                            )
                        nc.vector.tensor_tensor(
                            Vg[:, tci, 4 * dh : 4 * dh + 4, :DK],
                            ps.rearrange("p (h d) -> p h d", d=DK),
                            bv_r[:, dh * 512 : (dh + 1) * 512].rearrange(
                                "p (h d) -> p h d", d=DK
                            ),
                            mybir.AluOpType.add,
                        )

                # phase B: per head h: project Qt[h]/Kt[h], then the two
                # attention units, software-pipelined so attn@V of unit u-1
                # overlaps scores/exp of unit u (PT pool bufs=2).
                def proj_head(h, w_sb, b_p):
                    dst = qk.tile([P, T], BF, tag="qth" if w_sb is Wq else "kth")
                    for tt in range(T // 512):
                        ps = ps1.tile([P, 512], F32, tag="ps1")
                        for k in range(KO):
                            nc.tensor.matmul(
                                ps[:],
                                w_sb[:, k, h * P : (h + 1) * P],
                                Xt[:, k, tt * 512 : (tt + 1) * 512],
                                start=(k == 0),
                                stop=(k == KO - 1),
                            )
                        nc.vector.tensor_tensor(
                            dst[:, tt * 512 : (tt + 1) * 512],
                            ps[:],
                            b_p[:, h : h + 1].to_broadcast((P, 512)),
                            mybir.AluOpType.add,
                        )
                    return dst

                def scores_unit(h, ch, Qth, Kth):
                    chp = (1 - ch) if h < N_CROSS else ch  # kv channel
                    q0 = ch * N
                    m0 = chp * N
                    PT = pt_pool.tile([P, KO, N], BF, tag="pt")
                    for mi in range(KO):
                        for nh in range(2):
                            ps = ps_s.tile([P, 512], F32, tag="s")
                            nc.tensor.matmul(
                                ps[:],
                                Kth[:, m0 + mi * P : m0 + (mi + 1) * P],
                                Qth[:, q0 + nh * 512 : q0 + (nh + 1) * 512],
                                start=True,
                                stop=True,
                            )
                            nc.scalar.activation(
                                PT[:, mi, nh * 512 : (nh + 1) * 512],
                                ps[:],
                                mybir.ActivationFunctionType.Exp,
                                scale=SCALE,
                            )
                    return (PT, h, ch, chp, q0)

                def attnv_unit(state):
                    PT, h, ch, chp, q0 = state
                    for ni in range(KO):
                        psz = ps_z.tile([P, DK + 1], F32, tag="z")
                        for mi in range(KO):
                            nc.tensor.matmul(
                                psz[:],
                                PT[:, mi, ni * P : (ni + 1) * P],
                                Vg[:, chp * KO + mi, h, :],
                                start=(mi == 0),
                                stop=(mi == KO - 1),
                            )
                        r = att_sm.tile([P, 1], F32, tag="r")
                        nc.vector.reciprocal(r[:], psz[:, DK : DK + 1])
                        zn = att_sm.tile([P, DK], BF, tag="zn")
                        nc.vector.tensor_tensor(
                            zn[:],
                            psz[:, :DK],
                            r[:, 0:1].to_broadcast((P, DK)),
                            mybir.AluOpType.mult,
                        )
                        pzt = ps_zt.tile([P, P], BF, tag="zt")
                        nc.tensor.transpose(pzt[:], zn[:], ident[:])
                        nc.vector.tensor_copy(
                            Zt[:, h, q0 + ni * P : q0 + (ni + 1) * P], pzt[:]
                        )

                prev = None
                for h in range(H):
                    Qth = proj_head(h, Wq, bq_p)
                    Kth = proj_head(h, Wk, bk_p)
                    for ch in range(2):
                        cur = scores_unit(h, ch, Qth, Kth)
                        if prev is not None:
                            attnv_unit(prev)
                        prev = cur
                attnv_unit(prev)

            # ---- phase C: output projection ----
            with (
                tc.tile_pool(name="wo_pool", bufs=1) as wo_pool,
                tc.tile_pool(name="y_pool", bufs=4) as y_pool,
                tc.tile_pool(name="ps_y", bufs=4, space="PSUM") as ps_y,
            ):
                Wo = wo_pool.tile([P, KO, D], BF)
                wo_r = wo_d.rearrange("(o p) f -> p o f", p=P)
                for o in range(KO):
                    nc.sync.dma_start(Wo[:, o, :], wo_r[:, o, :])
                for tci in range(TC):
                    for dh in range(2):
                        ps = ps_y.tile([P, 512], F32, tag="y")
                        for k in range(KO):
                            nc.tensor.matmul(
                                ps[:],
                                Zt[:, k, tci * P : (tci + 1) * P],
                                Wo[:, k, dh * 512 : (dh + 1) * 512],
                                start=(k == 0),
                                stop=(k == KO - 1),
                            )
                        y = y_pool.tile([P, 512], F32, tag="y_sb")
                        nc.vector.tensor_tensor(
                            y[:],
                            ps[:],
                            bo_r[:, dh * 512 : (dh + 1) * 512],
                            mybir.AluOpType.add,
                        )
                        nc.sync.dma_start(
                            out_d[
                                tci * P : (tci + 1) * P,
                                dh * 512 : (dh + 1) * 512,
                            ],
                            y[:],
                        )
    return nc


def _get_program():
    if "nc" not in _CACHE:
        _legalize_install()
        _CACHE["nc"] = _build()
    return _CACHE["nc"]


def make_in_maps(inputs):
    x = np.asarray(inputs["x"], dtype=np.float32)
    bs2 = x.shape[0]
    n_cores = bs2 // 2
    bf = ml_dtypes.bfloat16

    weights = {
        name: np.ascontiguousarray(np.asarray(inputs[name], dtype=np.float32)).astype(
            bf
        )
        for name in ("Wq", "Wk", "Wv", "Wo")
    }
    biases = {
        name: np.ascontiguousarray(np.asarray(inputs[name], dtype=np.float32))
        for name in ("bq", "bk", "bv", "bo")
    }

    in_maps = []
    for c in range(n_cores):
        xt = np.ascontiguousarray(x[2 * c : 2 * c + 2].reshape(T, D).T).astype(bf)
        in_maps.append(
            {
                "xt": xt,
                "wq": weights["Wq"],
                "wk": weights["Wk"],
                "wv": weights["Wv"],
                "wo": weights["Wo"],
                "bq": biases["bq"],
                "bk": biases["bk"],
                "bv": biases["bv"],
                "bo": biases["bo"],
            }
        )
    return in_maps


def kernel(**inputs):
    bs2 = np.asarray(inputs["x"]).shape[0]
    n_cores = bs2 // 2
    in_maps = make_in_maps(inputs)
    nc = _get_program()
    res = run_bass_kernel_spmd(nc, in_maps, core_ids=list(range(n_cores)))
    out = np.empty((bs2, N, D), dtype=np.float32)
    for c in range(n_cores):
        out[2 * c : 2 * c + 2] = res.results[c]["out"].reshape(2, N, D)
    return out


# revision 15
# speedup vs baseline: 1.0296x; 1.0296x over previous
"""Cross-channel multi-head attention on 8 Trainium2 NeuronCores.

Sharding: data-parallel over the batch axis. bs2=16 sequences form bs=8
(batch, 2-channel) pairs; each core handles one pair fully locally
(cross-channel attention couples only the two channels of the same batch
element), so no collectives are needed.

Per core (T=2048 tokens = 2 channels x 1024 patches, D=1024, H=8 heads,
dk=128; heads 0..5 attend to the other channel's K/V, heads 6..7 to the
same channel):
  1. Qt = (x @ Wq + bq)^T and Kt likewise, in [D, T] layout (dk on
     partitions) -- exactly what the scores matmul wants as lhsT/rhs.
  2. V = x @ Wv + bv in natural [T, D] layout, stored per head with an
     extra ones column (softmax denominator trick).
  3. Per (head, channel): S^T[m,n] = Kt_h^T-slice x Qt_h-slice;
     P^T = exp(S^T / sqrt(dk)) (no max subtraction: |S|<~6 for this
     distribution); Z_aug = P^T.T @ [V_h | 1] so Z_aug[:,dk] is the
     denominator; Z = Z_aug[:,:dk] * (1/denom); PE-transpose Z into
     Zt [D, T].
  4. out = Zt^T-slices @ Wo + bo in natural [T, D] layout.

All matmuls in bf16 with f32 PSUM accumulation (~5e-3 rel err vs the f32
reference). The host pre-transposes/casts x to bf16 [D, T] per core and
casts the weights to bf16, so the device does no f32 transposes.
"""

import sys

if "/opt/trn_rl_repo" not in sys.path:
    sys.path.insert(0, "/opt/trn_rl_repo")

import numpy as np
import ml_dtypes

import concourse.bass as bass
import concourse.tile as tile
from concourse import mybir
from concourse.bass_utils import run_bass_kernel_spmd
from concourse.masks import make_identity

# Walrus in this container rejects >1 wait condition on TPB_CTRL ops
# (Tile's kernel-tail drain carries one per active proc). Split them.
import os

_here = os.path.dirname(os.path.abspath(__file__))
if _here not in sys.path:
    sys.path.insert(0, _here)
try:
    import bir_legalize
except ImportError:  # graded in a bare dir: fall back to inline copy
    bir_legalize = None

N = 1024  # patches per channel
D = 1024
H = 8
DK = 128
N_CROSS = 6
T = 2 * N  # tokens per core (2 channels of one batch element)
P = 128
KO = D // P  # 8 outer chunks of the 1024-wide dims
TC = T // P  # 16 token chunks
BF = mybir.dt.bfloat16
F32 = mybir.dt.float32
SCALE = 1.0 / float(np.sqrt(DK))

_CACHE = {}


def _legalize_install():
    if bir_legalize is not None:
        bir_legalize.install()
        return
    # Inline fallback (kernel.py must be self-contained when graded).
    import json
    import concourse.bass2jax as bass2jax
    from concourse.bass_utils import compile_bir_kernel as _orig

    if getattr(bass2jax.compile_bir_kernel, "_legalized", False):
        return

    OPCODE_MAX = {}
    SKIP = set()

    def _legalize(bir_json):
        d = json.loads(bir_json)
        changed = False
        for fn in d.get("functions", []):
            for bb in fn.get("blocks") or fn.get("basicblocks") or []:
                out = []
                for inst in bb.get("instructions", []):
                    sync = inst.get("sync_info") or {}
                    waits = sync.get("on_wait") or []
                    cap = OPCODE_MAX.get(inst.get("opcode"), 1)
                    if len(waits) > cap and inst.get("opcode") not in SKIP:
                        extra, keep = waits[:-cap], waits[-cap:]
                        for i, w in enumerate(extra):
                            out.append(
                                {
                                    "debug": inst.get("debug", 0),
                                    "engine": inst["engine"],
                                    "ins": [],
                                    "outs": [],
                                    "is_reset_sema": False,
                                    "name": f"{inst['name']}-sw{i}",
                                    "opcode": "Drain",
                                    "sync_info": {"on_update": [], "on_wait": [w]},
                                }
                            )
                        sync["on_wait"] = keep
                        inst["sync_info"] = sync
                        changed = True
                    out.append(inst)
                bb["instructions"] = out
        return json.dumps(d).encode() if changed else bir_json

    def compile_bir_kernel(bir_json, tmpdir, neff_name="file.neff"):
        return _orig(_legalize(bir_json), tmpdir, neff_name)

    compile_bir_kernel._legalized = True
    bass2jax.compile_bir_kernel = compile_bir_kernel


def _bcast_rows(ap, p):
    """Replicate a 1-D DRAM AP across p partitions (stride-0 partition dim)."""
    return bass.AP(tensor=ap.tensor, offset=ap.offset, ap=[[0, p], *ap.ap])


def _build():
    nc = bass.Bass()

    xt_d = nc.dram_tensor("xt", [D, T], BF, kind="ExternalInput").ap()
    wq_d = nc.dram_tensor("wq", [D, D], BF, kind="ExternalInput").ap()
    wk_d = nc.dram_tensor("wk", [D, D], BF, kind="ExternalInput").ap()
    wv_d = nc.dram_tensor("wv", [D, D], BF, kind="ExternalInput").ap()
    wo_d = nc.dram_tensor("wo", [D, D], BF, kind="ExternalInput").ap()
    bq_d = nc.dram_tensor("bq", [D], F32, kind="ExternalInput").ap()
    bk_d = nc.dram_tensor("bk", [D], F32, kind="ExternalInput").ap()
    bv_d = nc.dram_tensor("bv", [D], F32, kind="ExternalInput").ap()
    bo_d = nc.dram_tensor("bo", [D], F32, kind="ExternalInput").ap()
    out_d = nc.dram_tensor("out", [T, D], F32, kind="ExternalOutput").ap()

    with tile.TileContext(nc) as tc:
        with (
            tc.tile_pool(name="consts", bufs=1) as consts,
            tc.tile_pool(name="big", bufs=1) as big,
        ):
            ident = consts.tile([P, P], BF)
            make_identity(nc, ident)
            bq_p = consts.tile([P, KO], F32)
            nc.sync.dma_start(bq_p[:], bq_d.rearrange("(o p) -> p o", p=P))
            bk_p = consts.tile([P, KO], F32)
            nc.sync.dma_start(bk_p[:], bk_d.rearrange("(o p) -> p o", p=P))
            bv_r = consts.tile([P, D], F32)
            nc.sync.dma_start(bv_r[:], _bcast_rows(bv_d, P))
            bo_r = consts.tile([P, D], F32)
            nc.sync.dma_start(bo_r[:], _bcast_rows(bo_d, P))

            Vg = big.tile([P, TC, H, DK + 1], BF)  # natural V + ones col
            nc.vector.memset(Vg[:, :, :, DK : DK + 1], 1.0)
            Zt = big.tile([P, KO, T], BF)  # attention out, [dout, T]

            # ---- phases A (V proj) + B (QK proj fused with attention) ----
            with (
                tc.tile_pool(name="xt_w", bufs=1) as xt_w,
                tc.tile_pool(name="qk", bufs=2) as qk,
                tc.tile_pool(name="pt_pool", bufs=2) as pt_pool,
                tc.tile_pool(name="att_sm", bufs=4) as att_sm,
                tc.tile_pool(name="ps1", bufs=2, space="PSUM") as ps1,
                tc.tile_pool(name="ps_s", bufs=2, space="PSUM") as ps_s,
                tc.tile_pool(name="ps_z", bufs=2, space="PSUM") as ps_z,
                tc.tile_pool(name="ps_zt", bufs=2, space="PSUM") as ps_zt,
            ):
                # Warm the PE HAM clock gate with throwaway accumulating
                # matmul groups (dense, no psum rotation stalls) so the
                # first real matmuls run at 2.4 GHz instead of 1.2 GHz.
                warm_in = att_sm.tile([P, P], BF, tag="warm")
                nc.vector.memset(warm_in[:], 0.0)
                warm_rhs = att_sm.tile([P, 512], BF, tag="warm_rhs")
                nc.vector.memset(warm_rhs[:], 0.0)
                for g in range(2):
                    wps = ps_s.tile([P, 512], F32, tag="s")
                    for k in range(12):
                        nc.tensor.matmul(
                            wps[:],
                            warm_in[:],
                            warm_rhs[:],
                            start=(k == 0),
                            stop=(k == 11),
                        )

                # DMA: V-projection inputs (Xt, Wv) issue on sync, Wq/Wk on
                # gpsimd in parallel (descriptor issue is the startup
                # bottleneck). First k-chunks split fine across queues so the
                # first accumulation group can start ASAP.
                Xt = xt_w.tile([P, KO, T], BF)
                Wq = xt_w.tile([P, KO, D], BF)
                Wk = xt_w.tile([P, KO, D], BF)
                Wv = xt_w.tile([P, KO, D], BF)
                xt_r = xt_d.rearrange("(o p) t -> p o t", p=P)
                wq_r = wq_d.rearrange("(o p) f -> p o f", p=P)
                wk_r = wk_d.rearrange("(o p) f -> p o f", p=P)
                wv_r = wv_d.rearrange("(o p) f -> p o f", p=P)
                for q in range(4):
                    nc.sync.dma_start(
                        Xt[:, 0, q * 512 : (q + 1) * 512],
                        xt_r[:, 0, q * 512 : (q + 1) * 512],
                    )
                nc.sync.dma_start(Wv[:, 0, :512], wv_r[:, 0, :512])
                nc.sync.dma_start(Wv[:, 0, 512:], wv_r[:, 0, 512:])
                for o in range(1, KO):
                    nc.sync.dma_start(Xt[:, o, :], xt_r[:, o, :])
                    nc.sync.dma_start(Wv[:, o, :], wv_r[:, o, :])
                for o in range(KO):
                    nc.gpsimd.dma_start(Wq[:, o, :], wq_r[:, o, :])
                    nc.gpsimd.dma_start(Wk[:, o, :], wk_r[:, o, :])

                # phase A: V natural = Xt-chunk.T @ Wv. Iterate k-OUTER
                # across 8 concurrent PSUM groups (borrowing every psum
                # pool's banks) so PE has a deep backlog while the input
                # DMAs are still streaming in k-chunk order.
                groups = [(tci, dh) for tci in range(TC) for dh in range(2)]
                gpools = [ps1, ps1, ps_s, ps_s, ps_z, ps_z, ps_zt, ps_zt]
                gtags = ["ps1", "ps1", "s", "s", "z", "z", "zt", "zt"]
                for base in range(0, len(groups), 8):
                    tiles = [
                        gpools[g].tile(
                            [P, 512], F32, tag=gtags[g], name=f"vps_{base}_{g}"
                        )
                        for g in range(8)
                    ]
                    for k in range(KO):
                        for g in range(8):
                            tci, dh = groups[base + g]
                            nc.tensor.matmul(
                                tiles[g][:],
                                Xt[:, k, tci * P : (tci + 1) * P],
                                Wv[:, k, dh * 512 : (dh + 1) * 512],
                                start=(k == 0),
                                stop=(k == KO - 1),
                            )
                    for g in range(8):
                        tci, dh = groups[base + g]
                        nc.vector.tensor_tensor(
                            Vg[:, tci, 4 * dh : 4 * dh + 4, :DK],
                            tiles[g].rearrange("p (h d) -> p h d", d=DK),
                            bv_r[:, dh * 512 : (dh + 1) * 512].rearrange(
                                "p (h d) -> p h d", d=DK
                            ),
                            mybir.AluOpType.add,
                        )

                # phase B: per head h: project Qt[h]/Kt[h], then the two
                # attention units, software-pipelined so attn@V of unit u-1
                # overlaps scores/exp of unit u (PT pool bufs=2).
                def proj_head(h, w_sb, b_p):
                    dst = qk.tile([P, T], BF, tag="qth" if w_sb is Wq else "kth")
                    for tt in range(T // 512):
                        ps = ps1.tile([P, 512], F32, tag="ps1")
                        for k in range(KO):
                            nc.tensor.matmul(
                                ps[:],
                                w_sb[:, k, h * P : (h + 1) * P],
                                Xt[:, k, tt * 512 : (tt + 1) * 512],
                                start=(k == 0),
                                stop=(k == KO - 1),
                            )
                        nc.vector.tensor_tensor(
                            dst[:, tt * 512 : (tt + 1) * 512],
                            ps[:],
                            b_p[:, h : h + 1].to_broadcast((P, 512)),
                            mybir.AluOpType.add,
                        )
                    return dst

                def scores_unit(h, ch, Qth, Kth):
                    chp = (1 - ch) if h < N_CROSS else ch  # kv channel
                    q0 = ch * N
                    m0 = chp * N
                    PT = pt_pool.tile([P, KO, N], BF, tag="pt")
                    for mi in range(KO):
                        for nh in range(2):
                            ps = ps_s.tile([P, 512], F32, tag="s")
                            nc.tensor.matmul(
                                ps[:],
                                Kth[:, m0 + mi * P : m0 + (mi + 1) * P],
                                Qth[:, q0 + nh * 512 : q0 + (nh + 1) * 512],
                                start=True,
                                stop=True,
                            )
                            nc.scalar.activation(
                                PT[:, mi, nh * 512 : (nh + 1) * 512],
                                ps[:],
                                mybir.ActivationFunctionType.Exp,
                                scale=SCALE,
                            )
                    return (PT, h, ch, chp, q0)

                def attnv_unit(state):
                    PT, h, ch, chp, q0 = state
                    for ni in range(KO):
                        psz = ps_z.tile([P, DK + 1], F32, tag="z")
                        for mi in range(KO):
                            nc.tensor.matmul(
                                psz[:],
                                PT[:, mi, ni * P : (ni + 1) * P],
                                Vg[:, chp * KO + mi, h, :],
                                start=(mi == 0),
                                stop=(mi == KO - 1),
                            )
                        r = att_sm.tile([P, 1], F32, tag="r")
                        nc.vector.reciprocal(r[:], psz[:, DK : DK + 1])
                        zn = att_sm.tile([P, DK], BF, tag="zn")
                        nc.vector.tensor_tensor(
                            zn[:],
                            psz[:, :DK],
                            r[:, 0:1].to_broadcast((P, DK)),
                            mybir.AluOpType.mult,
                        )
                        pzt = ps_zt.tile([P, P], BF, tag="zt")
                        nc.tensor.transpose(pzt[:], zn[:], ident[:])
                        nc.vector.tensor_copy(
                            Zt[:, h, q0 + ni * P : q0 + (ni + 1) * P], pzt[:]
                        )

                prev = None
                for h in range(H):
                    Qth = proj_head(h, Wq, bq_p)
                    Kth = proj_head(h, Wk, bk_p)
                    for ch in range(2):
                        cur = scores_unit(h, ch, Qth, Kth)
                        if prev is not None:
                            attnv_unit(prev)
                        prev = cur
                attnv_unit(prev)

            # ---- phase C: output projection ----
            with (
                tc.tile_pool(name="wo_pool", bufs=1) as wo_pool,
                tc.tile_pool(name="y_pool", bufs=4) as y_pool,
                tc.tile_pool(name="ps_y", bufs=4, space="PSUM") as ps_y,
            ):
                Wo = wo_pool.tile([P, KO, D], BF)
                wo_r = wo_d.rearrange("(o p) f -> p o f", p=P)
                for o in range(KO):
                    nc.sync.dma_start(Wo[:, o, :], wo_r[:, o, :])
                for tci in range(TC):
                    for dh in range(2):
                        ps = ps_y.tile([P, 512], F32, tag="y")
                        for k in range(KO):
                            nc.tensor.matmul(
                                ps[:],
                                Zt[:, k, tci * P : (tci + 1) * P],
                                Wo[:, k, dh * 512 : (dh + 1) * 512],
                                start=(k == 0),
                                stop=(k == KO - 1),
                            )
                        y = y_pool.tile([P, 512], F32, tag="y_sb")
                        nc.vector.tensor_tensor(
                            y[:],
                            ps[:],
                            bo_r[:, dh * 512 : (dh + 1) * 512],
                            mybir.AluOpType.add,
                        )
                        nc.sync.dma_start(
                            out_d[
                                tci * P : (tci + 1) * P,
                                dh * 512 : (dh + 1) * 512,
                            ],
                            y[:],
                        )
    return nc


def _get_program():
    if "nc" not in _CACHE:
        _legalize_install()
        _CACHE["nc"] = _build()
    return _CACHE["nc"]


def make_in_maps(inputs):
    x = np.asarray(inputs["x"], dtype=np.float32)
    bs2 = x.shape[0]
    n_cores = bs2 // 2
    bf = ml_dtypes.bfloat16

    weights = {
        name: np.ascontiguousarray(np.asarray(inputs[name], dtype=np.float32)).astype(
            bf
        )
        for name in ("Wq", "Wk", "Wv", "Wo")
    }
    biases = {
        name: np.ascontiguousarray(np.asarray(inputs[name], dtype=np.float32))
        for name in ("bq", "bk", "bv", "bo")
    }

    in_maps = []
    for c in range(n_cores):
        xt = np.ascontiguousarray(x[2 * c : 2 * c + 2].reshape(T, D).T).astype(bf)
        in_maps.append(
            {
                "xt": xt,
                "wq": weights["Wq"],
                "wk": weights["Wk"],
                "wv": weights["Wv"],
                "wo": weights["Wo"],
                "bq": biases["bq"],
                "bk": biases["bk"],
                "bv": biases["bv"],
                "bo": biases["bo"],
            }
        )
    return in_maps


def kernel(**inputs):
    bs2 = np.asarray(inputs["x"]).shape[0]
    n_cores = bs2 // 2
    in_maps = make_in_maps(inputs)
    nc = _get_program()
    res = run_bass_kernel_spmd(nc, in_maps, core_ids=list(range(n_cores)))
    out = np.empty((bs2, N, D), dtype=np.float32)
    for c in range(n_cores):
        out[2 * c : 2 * c + 2] = res.results[c]["out"].reshape(2, N, D)
    return out
